# revision 1
# baseline (speedup 1.0000x reference)
# Causal self-attention on 8 TRN2 NeuronCores.
#
# Sharding (data + tensor parallel per the hint):
#   core c -> batch b = c // 4, head group g = c % 4 (4 heads of 64 dims = 256).
#   Wq/Wk/Wv are split column-wise (rows of W, since y = x @ W.T) per head
#   group; Wo is split row-wise. Each core computes a partial [S, D] output
#   (transposed on device as [D, S]); the host sums the 4 partials per batch
#   element (the "all-reduce" of row-parallel sharding) and transposes back.
#
# Device kernel (per core), all matmuls in fp32r (full-rate PE):
#   xT [D, S] resident in SBUF.
#   QT/KT [d'=256, S] = W x + b   (head dim on partitions; 1/8 scale folded
#                                  into Wq/bq on the host)
#   V    [S, d'=256]              (sequence on partitions)
#   per head pair (row-packed K=64 matmuls) and q-block of 512:
#     scoresT [k,q] = KT.T-free matmul; exp on ACT (no max subtraction --
#     inputs are N(0,1)-ish so scores are O(+-8) and exp is safe in fp32);
#     causal: skip fully-masked k-chunks, mask the 128x128 diagonal triangle;
#     PV accumulates [O; rowsum] over k-chunks via a ones-augmented V;
#     normalize via K=1 PE broadcast of the sums + DVE approx reciprocal
#     (gpsimd partition_broadcast is broken on HW; DVE is lane-aligned).
#   partialT [D, S] = WoT.T-free matmul over d' chunks, + bo (only on g==0
#   cores), DMA'd out.

import os

import numpy as np

S = 2048
D = 1024
DL = 256  # local head dims (4 heads x 64)
NCORES = 8

_cache = {}
LAST_EXEC_TIME_NS = None
LAST_TRACE_PATH = None


DEBUG = os.environ.get("KERNEL_DEBUG", "0") == "1"


def _build_bass():
    from concourse import bacc
    import concourse.tile as tile
    import concourse.mybir as mybir
    from concourse.bass import ts, ds

    f32 = mybir.dt.float32
    f32r = mybir.dt.float32r
    bf16 = mybir.dt.bfloat16
    Exp = mybir.ActivationFunctionType.Exp
    ADD = mybir.AluOpType.add

    nc = bacc.Bacc("TRN2", target_bir_lowering=False, debug=False)

    xT_d = nc.dram_tensor("xT", [D, S], f32r, kind="ExternalInput")
    wqT_d = nc.dram_tensor("wqT", [D, DL], f32r, kind="ExternalInput")
    wkT_d = nc.dram_tensor("wkT", [D, DL], f32r, kind="ExternalInput")
    wvT_d = nc.dram_tensor("wvT", [D, DL], f32r, kind="ExternalInput")
    woT_d = nc.dram_tensor("woT", [DL, D], f32r, kind="ExternalInput")
    bq_d = nc.dram_tensor("bq", [1, DL], f32r, kind="ExternalInput")
    bk_d = nc.dram_tensor("bk", [1, DL], f32r, kind="ExternalInput")
    bv_d = nc.dram_tensor("bv", [1, DL], f32r, kind="ExternalInput")
    bo_d = nc.dram_tensor("bo", [128, 8], f32, kind="ExternalInput")
    mask_d = nc.dram_tensor("mask", [128, 128], f32r, kind="ExternalInput")
    onesr_d = nc.dram_tensor("onesr", [128, 512], f32r, kind="ExternalInput")
    onesv_d = nc.dram_tensor("onesv", [128, 16, 4, 1], f32r, kind="ExternalInput")
    out_d = nc.dram_tensor("outT", [D, S], f32, kind="ExternalOutput")
    warm_d = nc.dram_tensor("warm", [1, 512], f32, kind="ExternalOutput")
    if DEBUG:
        qT_o = nc.dram_tensor("qT_o", [128, 2, S], f32r, kind="ExternalOutput")
        kT_o = nc.dram_tensor("kT_o", [128, 2, S], f32r, kind="ExternalOutput")
        v4_o = nc.dram_tensor("v4_o", [128, 16, 4, 65], f32r, kind="ExternalOutput")
        oT_o = nc.dram_tensor("oT_o", [128, 2, S], f32r, kind="ExternalOutput")

    with tile.TileContext(nc) as tc:
        with (
            tc.tile_pool(name="persist", bufs=1) as persist,
            tc.tile_pool(name="ptp", bufs=4) as ptp,
            tc.tile_pool(name="sup", bufs=2) as sup,
            tc.tile_pool(name="oup", bufs=2) as oup,
            tc.tile_pool(name="rbp", bufs=2) as rbp,
            tc.tile_pool(name="stp", bufs=2) as stp,
            tc.tile_pool(name="tbp", bufs=1) as tbp,
            tc.tile_pool(name="sc2", bufs=2, space="PSUM") as sc2,
            tc.tile_pool(name="mm", bufs=2, space="PSUM") as mm,
            tc.tile_pool(name="po", bufs=2, space="PSUM") as po,
        ):
            # ---- persistent SBUF tensors ----
            xT = persist.tile([128, 8, S], f32r, name="xT_sb")
            wqT = persist.tile([128, 8, DL], f32r, name="wqT_sb")
            wkT = persist.tile([128, 8, DL], f32r, name="wkT_sb")
            wvT = persist.tile([128, 8, DL], f32r, name="wvT_sb")
            woT = persist.tile([128, 2, D], f32r, name="woT_sb")
            bq = persist.tile([1, DL], f32r, name="bq_sb")
            bk = persist.tile([1, DL], f32r, name="bk_sb")
            bv = persist.tile([1, DL], f32r, name="bv_sb")
            bo = persist.tile([128, 8], f32, name="bo_sb")
            mask = persist.tile([128, 128], f32r, name="mask_sb")
            ones = persist.tile([128, 512], f32r, name="ones_sb")
            ones_bf = persist.tile([128, 512], bf16, name="ones_bf")
            qT = persist.tile([128, 2, S], f32r, name="qT_sb")
            kT = persist.tile([128, 2, S], f32r, name="kT_sb")
            v4 = persist.tile([128, 16, 4, 65], f32r, name="v4_sb")
            oT = persist.tile([128, 2, S], f32r, name="oT_sb")

            # ---- input DMAs (small first, then x chunk-wise) ----
            wq_r = wqT_d.ap().rearrange("(o p) f -> p o f", p=128)
            wk_r = wkT_d.ap().rearrange("(o p) f -> p o f", p=128)
            wv_r = wvT_d.ap().rearrange("(o p) f -> p o f", p=128)
            wo_r = woT_d.ap().rearrange("(o p) f -> p o f", p=128)
            x_r = xT_d.ap().rearrange("(o p) f -> p o f", p=128)
            nc.vector.memset(ones_bf[:], 1.0)
            nc.sync.dma_start(ones[:], onesr_d.ap())
            nc.sync.dma_start(wqT[:], wq_r)
            nc.scalar.dma_start(wkT[:], wk_r)
            nc.scalar.dma_start(wvT[:], wv_r)
            nc.sync.dma_start(bq[:], bq_d.ap())
            nc.sync.dma_start(bk[:], bk_d.ap())
            nc.sync.dma_start(bv[:], bv_d.ap())
            nc.sync.dma_start(bo[:], bo_d.ap())
            nc.sync.dma_start(mask[:], mask_d.ap())
            nc.sync.dma_start(v4[:, :, :, 64:65], onesv_d.ap())
            for tb in range(4):
                for mc in range(8):
                    eng = nc.sync if mc % 2 == 0 else nc.scalar
                    eng.dma_start(
                        xT[:, mc, ts(tb, 512)], x_r[:, mc, ts(tb, 512)]
                    )
                if tb == 0:
                    nc.scalar.dma_start(woT[:], wo_r)

            psW = sc2.tile([128, 2, 512], f32, tag="sc", name="psW")
            for i in range(128):
                nc.tensor.matmul(
                    psW[:, i % 2, :],
                    lhsT=ones_bf[:, 0:128],
                    rhs=ones_bf[:],
                    start=True,
                    stop=True,
                    skip_group_check=True,
                )
            wstg = stp.tile([1, 512], f32, tag="wst", name="wstg", bufs=1)
            nc.vector.tensor_copy(wstg[:], psW[0:1, 0, :])
            nc.sync.dma_start(warm_d.ap(), wstg[:])

            def proj_qk(wsb, bsb, dst, t, qb):
                ps = mm.tile([128, 512], f32, tag="mm")
                for mc in range(8):
                    nc.tensor.matmul(
                        ps,
                        lhsT=wsb[:, mc, ts(t, 128)],
                        rhs=xT[:, mc, ts(qb, 512)],
                        start=(mc == 0),
                        stop=False,
                    )
                nc.tensor.matmul(
                    ps,
                    lhsT=bsb[:, ts(t, 128)],
                    rhs=ones[0:1, :],
                    start=False,
                    stop=True,
                )
                nc.vector.tensor_copy(dst[:, t, ts(qb, 512)], ps)

            def proj_v(st):
                ps = mm.tile([128, 512], f32, tag="mm")
                psv = ps[:, 0:256]
                for mc in range(8):
                    nc.tensor.matmul(
                        psv,
                        lhsT=xT[:, mc, ts(st, 128)],
                        rhs=wvT[:, mc, :],
                        start=(mc == 0),
                        stop=False,
                    )
                nc.tensor.matmul(
                    psv,
                    lhsT=ones[0:1, 0:128],
                    rhs=bv[:],
                    start=False,
                    stop=True,
                )
                nc.vector.tensor_copy(
                    v4[:, st, :, 0:64], psv.rearrange("p (h d) -> p h d", h=4)
                )

            def attn_block(pair, qb, fill=None, fill_every=1):
                # heads (2*pair, 2*pair+1); q columns [512*qb, 512*qb+512)
                psA = po.tile([128, 512], f32, tag="po")
                psB = po.tile([128, 512], f32, tag="po")
                nchunks = 4 * qb + 4
                for c in range(nchunks):
                    if fill and c % fill_every == fill_every - 1:
                        fill.pop(0)()
                    dc = c - 4 * qb
                    q0 = 128 * dc if dc >= 0 else 0
                    w = 512 - q0
                    first = c == 0
                    last = c == nchunks - 1
                    ps2 = sc2.tile([128, 2, 512], f32, tag="sc")
                    for hh in (0, 1):
                        prow = slice(64 * hh, 64 * hh + 64)
                        nc.tensor.matmul(
                            ps2[:, hh, :w],
                            lhsT=kT[prow, pair, ts(c, 128)],
                            rhs=qT[prow, pair, ds(512 * qb + q0, w)],
                            start=True,
                            stop=True,
                        )
                    pt = ptp.tile([128, 2, 512], f32r, tag="pt")
                    nc.scalar.activation(pt[:, :, :w], ps2[:, :, :w], Exp)
                    if dc >= 0:
                        nc.vector.tensor_mul(
                            pt[:, :, 0:128],
                            pt[:, :, 0:128],
                            mask[:, None, :].to_broadcast((128, 2, 128)),
                        )
                    for hh, psO in ((0, psA), (1, psB)):
                        nc.tensor.matmul(
                            psO[0:65, ds(q0, w)],
                            lhsT=v4[:, c, 2 * pair + hh, :],
                            rhs=pt[:, hh, :w],
                            start=first,
                            stop=last,
                            skip_group_check=True,
                        )
                # normalization: sums -> SBUF(f32r) -> PE broadcast -> approx
                # reciprocal (PSUM -> SBUF) -> multiply
                sums = sup.tile([65, 1024], f32r, tag="su")
                nc.vector.tensor_copy(sums[64:65, 0:512], psA[64:65, :])
                nc.vector.tensor_copy(sums[64:65, 512:1024], psB[64:65, :])
                oUA = oup.tile([64, 512], f32, tag="ou")
                oUB = oup.tile([64, 512], f32, tag="ou")
                nc.vector.tensor_copy(oUA[:, :], psA[0:64, :])
                nc.vector.tensor_copy(oUB[:, :], psB[0:64, :])
                psR = mm.tile([128, 512], f32, tag="mm")
                nc.tensor.matmul(
                    psR[0:64, :],
                    lhsT=ones[64:65, 0:64],
                    rhs=sums[64:65, 0:512],
                    start=True,
                    stop=True,
                )
                psR2 = mm.tile([128, 512], f32, tag="mm")
                nc.tensor.matmul(
                    psR2[0:64, :],
                    lhsT=ones[64:65, 0:64],
                    rhs=sums[64:65, 512:1024],
                    start=True,
                    stop=True,
                )
                rbA = rbp.tile([64, 512], f32, tag="rb")
                rbB = rbp.tile([64, 512], f32, tag="rb")
                nc.vector.reciprocal_approx_fast(rbA[:, :], psR[0:64, :])
                nc.vector.reciprocal_approx_fast(rbB[:, :], psR2[0:64, :])
                tmpB = tbp.tile([64, 512], f32r, tag="tb")
                nc.vector.tensor_mul(
                    oT[0:64, pair, ts(qb, 512)], oUA[:, :], rbA[:, :]
                )
                nc.vector.tensor_mul(tmpB[:, :], oUB[:, :], rbB[:, :])
                nc.scalar.dma_start(oT[64:128, pair, ts(qb, 512)], tmpB[:, :])

            def out_proj_jt(jt, sb):
                    ps = mm.tile([128, 512], f32, tag="mm")
                    for dchunk in range(2):
                        nc.tensor.matmul(
                            ps,
                            lhsT=woT[:, dchunk, ts(jt, 128)],
                            rhs=oT[:, dchunk, ts(sb, 512)],
                            start=(dchunk == 0),
                            stop=(dchunk == 1),
                        )
                    stg = stp.tile([128, 512], f32, tag="st")
                    nc.vector.tensor_tensor(
                        stg[:],
                        ps,
                        bo[:, jt : jt + 1].to_broadcast((128, 512)),
                        ADD,
                    )
                    nc.sync.dma_start(out_d.ap()[ts(jt, 128), ts(sb, 512)], stg[:])

            def out_proj(sb):
                for jt in range(8):
                    out_proj_jt(jt, sb)

            # software-pipelined emission: per q-block wave, produce the
            # projections it needs, then attention, then the output slice
            def emit_A(qb):
                for t in range(2):
                    proj_qk(wqT, bq, qT, t, qb)
                for st in range(4 * qb, 4 * qb + 4):
                    proj_v(st)
                for t in range(2):
                    proj_qk(wkT, bk, kT, t, qb)

            emit_A(0)
            for qb in range(4):
                ath = []
                if qb < 3:
                    nxt = qb + 1
                    for t in range(2):
                        ath.append(
                            lambda t=t, nxt=nxt: proj_qk(wqT, bq, qT, t, nxt)
                        )
                    for st in range(4 * nxt, 4 * nxt + 4):
                        ath.append(lambda st=st: proj_v(st))
                    for t in range(2):
                        ath.append(
                            lambda t=t, nxt=nxt: proj_qk(wkT, bk, kT, t, nxt)
                        )
                cth = []
                if qb == 1:
                    cth = [
                        lambda jt=jt: out_proj_jt(jt, 0) for jt in range(8)
                    ]
                elif qb == 2:
                    cth = [
                        lambda jt=jt: out_proj_jt(jt, 1) for jt in range(4)
                    ]
                elif qb == 3:
                    cth = [
                        lambda jt=jt: out_proj_jt(jt + 4, 1) for jt in range(4)
                    ] + [
                        lambda jt=jt: out_proj_jt(jt, 2) for jt in range(8)
                    ]
                thunks = []
                for i in range(max(len(ath), len(cth))):
                    if i < len(ath):
                        thunks.append(ath[i])
                    if i < len(cth):
                        thunks.append(cth[i])
                fe = max(1, (2 * (4 * qb + 4)) // (len(thunks) + 1))
                attn_block(0, qb, fill=thunks, fill_every=fe)
                attn_block(1, qb, fill=thunks, fill_every=fe)
                for th in thunks:
                    th()
            out_proj(3)
            if DEBUG:
                nc.sync.dma_start(qT_o.ap(), qT[:])
                nc.sync.dma_start(kT_o.ap(), kT[:])
                nc.sync.dma_start(v4_o.ap(), v4[:])
                nc.sync.dma_start(oT_o.ap(), oT[:])

    nc.compile()
    return nc


def _get_bass():
    if "nc" not in _cache:
        _cache["nc"] = _build_bass()
    return _cache["nc"]


def _shard_inputs(x, Wq, bq, Wk, bk, Wv, bv, Wo, bo):
    x = np.asarray(x, dtype=np.float32)
    Wq = np.asarray(Wq, dtype=np.float32)
    Wk = np.asarray(Wk, dtype=np.float32)
    Wv = np.asarray(Wv, dtype=np.float32)
    Wo = np.asarray(Wo, dtype=np.float32)
    bq = np.asarray(bq, dtype=np.float32)
    bk = np.asarray(bk, dtype=np.float32)
    bv = np.asarray(bv, dtype=np.float32)
    bo = np.asarray(bo, dtype=np.float32)

    kk = np.arange(128)[:, None]
    qq = np.arange(128)[None, :]
    mask128 = (kk <= qq).astype(np.float32)
    bo_sb = np.ascontiguousarray(bo.reshape(8, 128).T)
    bo_zero = np.zeros_like(bo_sb)
    onesr = np.ones((128, 512), np.float32)
    onesv = np.ones((128, 16, 4, 1), np.float32)

    xT = [np.ascontiguousarray(x[b].T) for b in range(x.shape[0])]
    in_maps = []
    for c in range(NCORES):
        b, g = divmod(c, 4)
        sl = slice(DL * g, DL * (g + 1))
        in_maps.append(
            {
                "xT": xT[b],
                "wqT": np.ascontiguousarray(Wq[sl].T) * 0.125,
                "wkT": np.ascontiguousarray(Wk[sl].T),
                "wvT": np.ascontiguousarray(Wv[sl].T),
                "woT": np.ascontiguousarray(Wo[:, sl].T),
                "bq": (bq[sl] * 0.125).reshape(1, DL),
                "bk": bk[sl].reshape(1, DL),
                "bv": bv[sl].reshape(1, DL),
                "bo": bo_sb if g == 0 else bo_zero,
                "mask": mask128,
                "onesr": onesr,
                "onesv": onesv,
            }
        )
    return in_maps


def kernel(x, Wq, bq, Wk, bk, Wv, bv, Wo, bo):
    global LAST_EXEC_TIME_NS, LAST_TRACE_PATH
    from concourse.bass_utils import run_bass_kernel_spmd

    nc = _get_bass()
    in_maps = _shard_inputs(x, Wq, bq, Wk, bk, Wv, bv, Wo, bo)

    trace = os.environ.get("KERNEL_TRACE", "0") == "1"
    res = run_bass_kernel_spmd(
        nc, in_maps, core_ids=list(range(NCORES)), trace=trace
    )
    LAST_EXEC_TIME_NS = res.exec_time_ns
    if res.instructions_and_trace is not None:
        LAST_TRACE_PATH = res.instructions_and_trace[1]

    B = 2
    out = np.empty((B, S, D), dtype=np.float32)
    for b in range(B):
        acc = res.results[4 * b]["outT"].astype(np.float32)
        for g in range(1, 4):
            acc = acc + res.results[4 * b + g]["outT"]
        out[b] = acc.T
    return out



# revision 5
# speedup vs baseline: 1.4316x; 1.4316x over previous
# Causal self-attention on 8 TRN2 NeuronCores.
#
# Sharding (data + tensor parallel per the hint):
#   core c -> batch b = c // 4, head group g = c % 4 (4 heads of 64 dims = 256).
#   Wq/Wk/Wv split column-wise per head group; Wo row-wise. Each core emits a
#   partial [D, S] output in bf16; the host sums the 4 partials per batch
#   (the "all-reduce" of row-parallel sharding), transposes, and adds
#   bo' = bo + bv @ Wo.T (the V-bias commutes through softmax-normalize +
#   out-projection, so it is folded into the host-side bias).
#
# Device kernel (per core), all matmuls bf16 (PE streams 1 col/cycle for
# every dtype, so bf16 costs the same PE time as fp32 but halves DMA/SBUF
# and unlocks 2x DVE modes; tolerance is 2e-2, bf16 lands ~1e-3):
#   xT [D, S] resident in SBUF (bf16).
#   QT/KT [128 = 2 heads x 64, pair, S] = W x; bias added on DVE during the
#     PSUM->SBUF stage via per-partition tensor_scalar_add (no bias matmuls;
#     1/8 softmax scale folded into Wq/bq on the host).
#   V [S, 4 heads, 64+1] with a ones column (rowsum rides along in PV).
#   Attention is one global chunk pipeline across all (pair, q-block)
#   blocks: per chunk, a row-tiled pair of K=64 score matmuls (two heads
#   concurrently in the PE array), exp on ACT (PSUM->SBUF bf16), causal
#   mask multiply on GpSimd (diag chunks), then PV accumulation.
#   The PE stream is emitted with scores lookahead 1 (scores of chunk i+1
#   precede PV of chunk i) and a calibrated amount of "filler" matmuls
#   (projections for later blocks + out-projection of finished q-blocks)
#   between them, so the PE never idles waiting for ACT and the HAM clock
#   gate stays at 2.4 GHz. ACT's exp stream is the secondary resource
#   (~68us vs ~100us PE); fillers are deferred so the late, ACT-heavy
#   blocks still have PE work available.
#   Normalization: rowsums -> PE broadcast (K=1 matmul) -> DVE approx
#   reciprocal -> multiply (head 0 on DVE in place, head 1 via tmp + DMA to
#   partitions 64..127).
#   Out projection: 2 accumulating matmuls per [128,512] tile, staged
#   PSUM->SBUF bf16 alternating DVE/ACT, DMA'd out bf16 (no bias on device).

import os

import numpy as np

S = 2048
D = 1024
DL = 256  # local head dims (4 heads x 64)
NCORES = 8

_cache = {}
LAST_EXEC_TIME_NS = None
LAST_TRACE_PATH = None


DEBUG = os.environ.get("KERNEL_DEBUG", "0") == "1"


def _build_bass():
    from concourse import bacc
    import concourse.tile as tile
    import concourse.mybir as mybir
    from concourse.bass import ts, ds

    f32 = mybir.dt.float32
    bf16 = mybir.dt.bfloat16
    Exp = mybir.ActivationFunctionType.Exp

    nc = bacc.Bacc("TRN2", target_bir_lowering=False, debug=False)

    xT_d = nc.dram_tensor("xT", [D, S], bf16, kind="ExternalInput")
    wqT_d = nc.dram_tensor("wqT", [D, DL], bf16, kind="ExternalInput")
    wkT_d = nc.dram_tensor("wkT", [D, DL], bf16, kind="ExternalInput")
    wvT_d = nc.dram_tensor("wvT", [D, DL], bf16, kind="ExternalInput")
    woT_d = nc.dram_tensor("woT", [DL, D], bf16, kind="ExternalInput")
    bq_d = nc.dram_tensor("bq", [128, 2], f32, kind="ExternalInput")
    bk_d = nc.dram_tensor("bk", [128, 2], f32, kind="ExternalInput")
    mask_d = nc.dram_tensor("mask", [128, 128], bf16, kind="ExternalInput")
    onesv_d = nc.dram_tensor("onesv", [128, 16, 4, 1], bf16, kind="ExternalInput")
    out_d = nc.dram_tensor("outT", [D, S], bf16, kind="ExternalOutput")
    warm_d = nc.dram_tensor("warm", [2, 512], f32, kind="ExternalOutput")
    if DEBUG:
        qT_o = nc.dram_tensor("qT_o", [128, 2, S], bf16, kind="ExternalOutput")
        kT_o = nc.dram_tensor("kT_o", [128, 2, S], bf16, kind="ExternalOutput")
        v4_o = nc.dram_tensor("v4_o", [128, 16, 4, 65], bf16, kind="ExternalOutput")
        oT_o = nc.dram_tensor("oT_o", [128, 2, S], bf16, kind="ExternalOutput")

    with tile.TileContext(nc) as tc:
        with (
            tc.tile_pool(name="persist", bufs=1) as persist,
            tc.tile_pool(name="ptp", bufs=4) as ptp,
            tc.tile_pool(name="oup", bufs=2) as oup,
            tc.tile_pool(name="rbp", bufs=2) as rbp,
            tc.tile_pool(name="stp", bufs=3) as stp,
            tc.tile_pool(name="tbp", bufs=2) as tbp,
            tc.tile_pool(name="wsp", bufs=1) as wsp,
            tc.tile_pool(name="sc2", bufs=2, space="PSUM") as sc2,
            tc.tile_pool(name="mm", bufs=2, space="PSUM") as mm,
            tc.tile_pool(name="po", bufs=2, space="PSUM") as po,
        ):
            # ---- persistent SBUF tensors ----
            xT = persist.tile([128, 8, S], bf16, name="xT_sb")
            wqT = persist.tile([128, 8, DL], bf16, name="wqT_sb")
            wkT = persist.tile([128, 8, DL], bf16, name="wkT_sb")
            wvT = persist.tile([128, 8, DL], bf16, name="wvT_sb")
            woT = persist.tile([128, 2, D], bf16, name="woT_sb")
            bq = persist.tile([128, 2], f32, name="bq_sb")
            bk = persist.tile([128, 2], f32, name="bk_sb")
            mask = persist.tile([128, 128], bf16, name="mask_sb")
            ones_bf = persist.tile([128, 512], bf16, name="ones_bf")
            qT = persist.tile([128, 2, S], bf16, name="qT_sb")
            kT = persist.tile([128, 2, S], bf16, name="kT_sb")
            v4 = persist.tile([128, 16, 4, 65], bf16, name="v4_sb")
            oT = persist.tile([128, 2, S], bf16, name="oT_sb")

            # ---- input DMAs ----
            wq_r = wqT_d.ap().rearrange("(o p) f -> p o f", p=128)
            wk_r = wkT_d.ap().rearrange("(o p) f -> p o f", p=128)
            wv_r = wvT_d.ap().rearrange("(o p) f -> p o f", p=128)
            wo_r = woT_d.ap().rearrange("(o p) f -> p o f", p=128)
            x_r = xT_d.ap().rearrange("(o p) f -> p o f", p=128)
            nc.vector.memset(ones_bf[:], 1.0)
            # sync ring: wq first (gates the first projection), then x
            # quarter-by-quarter (q-block 0 first so compute starts early)
            nc.sync.dma_start(wqT[:], wq_r)
            for tb in range(4):
                nc.sync.dma_start(xT[:, :, ts(tb, 512)], x_r[:, :, ts(tb, 512)])
            # scalar ring: remaining weights + small constants
            nc.scalar.dma_start(wkT[:], wk_r)
            nc.scalar.dma_start(wvT[:], wv_r)
            nc.scalar.dma_start(bq[:], bq_d.ap())
            nc.scalar.dma_start(bk[:], bk_d.ap())
            nc.scalar.dma_start(mask[:], mask_d.ap())
            nc.scalar.dma_start(v4[:, :, :, 64:65], onesv_d.ap())
            nc.scalar.dma_start(woT[:], wo_r)

            # ---- ACT table preload: dummy exp while DMAs stream ----
            wexp = wsp.tile([1, 512], f32, name="wexp")
            nc.scalar.activation(wexp[:], ones_bf[0:1, :], Exp)
            nc.sync.dma_start(warm_d.ap()[1:2, :], wexp[:])

            # ---- PE warmup: keep the array busy (and HAM warming) until
            # the first projection's inputs arrive (~6-7us) ----
            NWARM = 26
            psW = mm.tile([128, 512], f32, tag="mm", name="psW")
            for i in range(NWARM):
                nc.tensor.matmul(
                    psW,
                    lhsT=ones_bf[:, 0:128],
                    rhs=ones_bf[:],
                    start=(i == 0),
                    stop=(i == NWARM - 1),
                    skip_group_check=True,
                )
            wstg = wsp.tile([1, 512], f32, name="wstg")
            nc.vector.tensor_copy(wstg[:], psW[0:1, :])
            nc.sync.dma_start(warm_d.ap()[0:1, :], wstg[:])

            # ---- filler units: (est_pe_ns, emit_fn) ----
            def qk_proj_units(wsb, bvec, dst, t, qb):
                cell = {}

                def mk(mc):
                    def fn():
                        if mc == 0:
                            cell["ps"] = mm.tile(
                                [128, 512], f32, tag="mm", name="psqk"
                            )
                        nc.tensor.matmul(
                            cell["ps"],
                            lhsT=wsb[:, mc, ts(t, 128)],
                            rhs=xT[:, mc, ts(qb, 512)],
                            start=(mc == 0),
                            stop=(mc == 7),
                            skip_group_check=True,
                        )
                        if mc == 7:
                            nc.vector.tensor_scalar_add(
                                dst[:, t, ts(qb, 512)],
                                cell["ps"],
                                bvec[:, t : t + 1],
                            )

                    return (270, fn)

                return [mk(mc) for mc in range(8)]

            def v_proj_units(st):
                cell = {}

                def mk(mc):
                    def fn():
                        if mc == 0:
                            cell["ps"] = mm.tile(
                                [128, 512], f32, tag="mm", name="psv"
                            )
                        nc.tensor.matmul(
                            cell["ps"][:, 0:256],
                            lhsT=xT[:, mc, ts(st, 128)],
                            rhs=wvT[:, mc, :],
                            start=(mc == 0),
                            stop=(mc == 7),
                            skip_group_check=True,
                        )
                        if mc == 7:
                            nc.vector.tensor_copy(
                                v4[:, st, :, 0:64],
                                cell["ps"][:, 0:256].rearrange(
                                    "p (h d) -> p h d", h=4
                                ),
                            )

                    return (160, fn)

                return [mk(mc) for mc in range(8)]

            op_count = [0]

            def outproj_units(sb):
                units = []
                for jt in range(8):

                    def fn(jt=jt):
                        ps = mm.tile([128, 512], f32, tag="mm", name="psop")
                        for dchunk in range(2):
                            nc.tensor.matmul(
                                ps,
                                lhsT=woT[:, dchunk, ts(jt, 128)],
                                rhs=oT[:, dchunk, ts(sb, 512)],
                                start=(dchunk == 0),
                                stop=(dchunk == 1),
                                skip_group_check=True,
                            )
                        stg = stp.tile([128, 512], bf16, tag="st", name="stg")
                        if op_count[0] % 2 == 0:
                            nc.vector.tensor_copy(stg[:], ps)
                        else:
                            nc.scalar.copy(stg[:], ps)
                        op_count[0] += 1
                        nc.sync.dma_start(
                            out_d.ap()[ts(jt, 128), ts(sb, 512)], stg[:]
                        )

                    units.append((560, fn))
                return units

            filler = []  # list of (cost, fn), consumed front-first
            consumed = [0]

            def drain(budget_ns):
                spent = 0
                while filler and spent < budget_ns:
                    cost, fn = filler.pop(0)
                    fn()
                    consumed[0] += 1
                    spent += cost

            def drain_until(count):
                # force-consume prerequisite units: a block's scores may
                # never be emitted into the PE FIFO ahead of the filler
                # matmuls that produce its Q/K/V (in-order queue deadlock)
                while filler and consumed[0] < count:
                    cost, fn = filler.pop(0)
                    fn()
                    consumed[0] += 1

            # ---- attention chunk pipeline ----
            class Ch:
                __slots__ = (
                    "pair", "qb", "c", "w", "q0", "dc",
                    "first", "last", "ps2", "pt",
                )

            chunks = []
            for qb in range(4):
                for pair in range(2):
                    nch = 4 * qb + 4
                    for c in range(nch):
                        ch = Ch()
                        ch.pair, ch.qb, ch.c = pair, qb, c
                        dc = c - 4 * qb
                        ch.dc = dc
                        ch.q0 = 128 * dc if dc >= 0 else 0
                        ch.w = 512 - ch.q0
                        ch.first = c == 0
                        ch.last = c == nch - 1
                        chunks.append(ch)

            def emit_scores(ch):
                ps2 = sc2.tile([128, 2, 512], f32, tag="sc", name="ps2")
                for hh in (0, 1):
                    prow = slice(64 * hh, 64 * hh + 64)
                    nc.tensor.matmul(
                        ps2[:, hh, : ch.w],
                        lhsT=kT[prow, ch.pair, ts(ch.c, 128)],
                        rhs=qT[prow, ch.pair, ds(512 * ch.qb + ch.q0, ch.w)],
                        start=True,
                        stop=True,
                    )
                ch.ps2 = ps2

            def emit_exp(ch):
                pt = ptp.tile([128, 2, 512], bf16, tag="pt", name="pt")
                nc.scalar.activation(pt[:, :, : ch.w], ch.ps2[:, :, : ch.w], Exp)
                if ch.dc >= 0:
                    nc.gpsimd.tensor_mul(
                        pt[:, :, 0:128],
                        pt[:, :, 0:128],
                        mask[:, None, :].to_broadcast((128, 2, 128)),
                    )
                ch.pt = pt

            blk = {}

            def emit_pv(ch):
                if ch.first:
                    blk["psA"] = po.tile([128, 512], f32, tag="po", name="psA")
                    blk["psB"] = po.tile([128, 512], f32, tag="po", name="psB")
                for hh, psO in ((0, blk["psA"]), (1, blk["psB"])):
                    nc.tensor.matmul(
                        psO[0:65, ds(ch.q0, ch.w)],
                        lhsT=v4[:, ch.c, 2 * ch.pair + hh, :],
                        rhs=ch.pt[:, hh, : ch.w],
                        start=ch.first,
                        stop=ch.last,
                        skip_group_check=True,
                    )

            def emit_norm(pair, qb):
                psA, psB = blk["psA"], blk["psB"]
                oA = oup.tile([128, 512], bf16, tag="ou", name="oA")
                oB = oup.tile([128, 512], bf16, tag="ou", name="oB")
                nc.vector.tensor_copy(oA[0:65, :], psA[0:65, :])
                nc.vector.tensor_copy(oB[0:65, :], psB[0:65, :])
                psR = mm.tile([128, 512], f32, tag="mm", name="psR")
                nc.tensor.matmul(
                    psR[0:64, :],
                    lhsT=ones_bf[64:65, 0:64],
                    rhs=oA[64:65, :],
                    start=True,
                    stop=True,
                    skip_group_check=True,
                )
                rbA = rbp.tile([128, 512], f32, tag="rb", name="rbA")
                nc.vector.reciprocal_approx_fast(rbA[0:64, :], psR[0:64, :])
                psR2 = mm.tile([128, 512], f32, tag="mm", name="psR2")
                nc.tensor.matmul(
                    psR2[0:64, :],
                    lhsT=ones_bf[64:65, 0:64],
                    rhs=oB[64:65, :],
                    start=True,
                    stop=True,
                    skip_group_check=True,
                )
                rbB = rbp.tile([128, 512], f32, tag="rb", name="rbB")
                nc.vector.reciprocal_approx_fast(rbB[0:64, :], psR2[0:64, :])
                nc.gpsimd.tensor_mul(
                    oT[0:64, pair, ts(qb, 512)], oA[0:64, :], rbA[0:64, :]
                )
                tmpB = tbp.tile([128, 512], bf16, tag="tb", name="tmpB")
                nc.gpsimd.tensor_mul(tmpB[0:64, :], oB[0:64, :], rbB[0:64, :])
                nc.sync.dma_start(oT[64:128, pair, ts(qb, 512)], tmpB[0:64, :])

            # ---- pre-phase: minimal projections for (pair0, qb0) ----
            for u in qk_proj_units(wqT, bq, qT, 0, 0):
                u[1]()
            for u in qk_proj_units(wkT, bk, kT, 0, 0):
                u[1]()
            for st in range(4):
                for u in v_proj_units(st):
                    u[1]()

            # ---- filler schedule (dependency-ordered, deliberately
            # back-loaded: projections just-in-time, out-proj deferred to
            # the late ACT-heavy blocks) ----
            # consumed during (p0,qb0): Q/K t1 qb0 (needed by p1,qb0)
            filler += qk_proj_units(wqT, bq, qT, 1, 0)
            filler += qk_proj_units(wkT, bk, kT, 1, 0)
            # during (p1,qb0): Q/K t0 qb1 + V st4-7 (needed by qb1)
            filler += qk_proj_units(wqT, bq, qT, 0, 1)
            filler += qk_proj_units(wkT, bk, kT, 0, 1)
            for st in range(4, 8):
                filler += v_proj_units(st)
            # during (p0,qb1): Q/K t1 qb1
            filler += qk_proj_units(wqT, bq, qT, 1, 1)
            filler += qk_proj_units(wkT, bk, kT, 1, 1)
            # during (p1,qb1): Q/K t0 qb2 + V st8-11
            filler += qk_proj_units(wqT, bq, qT, 0, 2)
            filler += qk_proj_units(wkT, bk, kT, 0, 2)
            for st in range(8, 12):
                filler += v_proj_units(st)
            # during (p0,qb2): Q/K t1 qb2
            filler += qk_proj_units(wqT, bq, qT, 1, 2)
            filler += qk_proj_units(wkT, bk, kT, 1, 2)
            # during (p1,qb2): Q/K t0 qb3 + V st12-15
            filler += qk_proj_units(wqT, bq, qT, 0, 3)
            filler += qk_proj_units(wkT, bk, kT, 0, 3)
            for st in range(12, 16):
                filler += v_proj_units(st)
            # during (p0,qb3): Q/K t1 qb3
            filler += qk_proj_units(wqT, bq, qT, 1, 3)
            filler += qk_proj_units(wkT, bk, kT, 1, 3)
            # out-proj units are appended as their q-block completes

            # units that must be consumed before each block's first scores
            # (cumulative position in the dependency-ordered filler list);
            # blocks in order (p,qb): (1,0),(0,1),(1,1),(0,2),(1,2),(0,3),(1,3)
            prereq = {
                (1, 0): 16,
                (0, 1): 64,
                (1, 1): 80,
                (0, 2): 128,
                (1, 2): 144,
                (0, 3): 192,
                (1, 3): 208,
            }

            emit_scores(chunks[0])
            emit_exp(chunks[0])
            for i, ch in enumerate(chunks):
                if i + 1 < len(chunks):
                    nxt = chunks[i + 1]
                    if nxt.first and (nxt.pair, nxt.qb) in prereq:
                        drain_until(prereq[(nxt.pair, nxt.qb)])
                    emit_scores(nxt)
                    emit_exp(nxt)
                # keep PE fed while ACT computes exp(ch)
                drain(300 + 0.45 * ch.w)
                emit_pv(ch)
                if ch.last:
                    emit_norm(ch.pair, ch.qb)
                    if ch.pair == 1:
                        filler += outproj_units(ch.qb)

            # tail: whatever out-proj work is left (at least sb=3)
            while filler:
                filler.pop(0)[1]()

            if DEBUG:
                nc.sync.dma_start(qT_o.ap(), qT[:])
                nc.sync.dma_start(kT_o.ap(), kT[:])
                nc.sync.dma_start(v4_o.ap(), v4[:])
                nc.sync.dma_start(oT_o.ap(), oT[:])

    nc.compile()
    return nc


def _get_bass():
    if "nc" not in _cache:
        _cache["nc"] = _build_bass()
    return _cache["nc"]


def _shard_inputs(x, Wq, bq, Wk, bk, Wv, bv, Wo, bo):
    import ml_dtypes

    bft = ml_dtypes.bfloat16
    x = np.asarray(x, dtype=np.float32)
    Wq = np.asarray(Wq, dtype=np.float32)
    Wk = np.asarray(Wk, dtype=np.float32)
    Wv = np.asarray(Wv, dtype=np.float32)
    Wo = np.asarray(Wo, dtype=np.float32)
    bq = np.asarray(bq, dtype=np.float32)
    bk = np.asarray(bk, dtype=np.float32)

    kk = np.arange(128)[:, None]
    qq = np.arange(128)[None, :]
    mask128 = (kk <= qq).astype(bft)
    onesv = np.ones((128, 16, 4, 1), bft)

    xT = [np.ascontiguousarray(x[b].T).astype(bft) for b in range(x.shape[0])]
    in_maps = []
    for c in range(NCORES):
        b, g = divmod(c, 4)
        sl = slice(DL * g, DL * (g + 1))
        in_maps.append(
            {
                "xT": xT[b],
                "wqT": (np.ascontiguousarray(Wq[sl].T) * 0.125).astype(bft),
                "wkT": np.ascontiguousarray(Wk[sl].T).astype(bft),
                "wvT": np.ascontiguousarray(Wv[sl].T).astype(bft),
                "woT": np.ascontiguousarray(Wo[:, sl].T).astype(bft),
                "bq": np.ascontiguousarray(
                    (bq[sl] * 0.125).reshape(2, 128).T
                ),
                "bk": np.ascontiguousarray(bk[sl].reshape(2, 128).T),
                "mask": mask128,
                "onesv": onesv,
            }
        )
    return in_maps


def kernel(x, Wq, bq, Wk, bk, Wv, bv, Wo, bo):
    global LAST_EXEC_TIME_NS, LAST_TRACE_PATH
    from concourse.bass_utils import run_bass_kernel_spmd

    nc = _get_bass()
    in_maps = _shard_inputs(x, Wq, bq, Wk, bk, Wv, bv, Wo, bo)

    trace = os.environ.get("KERNEL_TRACE", "0") == "1"
    res = run_bass_kernel_spmd(
        nc, in_maps, core_ids=list(range(NCORES)), trace=trace
    )
    LAST_EXEC_TIME_NS = res.exec_time_ns
    if res.instructions_and_trace is not None:
        LAST_TRACE_PATH = res.instructions_and_trace[1]

    bo_full = (
        np.asarray(bo, np.float64)
        + np.asarray(bv, np.float64) @ np.asarray(Wo, np.float64).T
    ).astype(np.float32)

    B = 2
    out = np.empty((B, S, D), dtype=np.float32)
    for b in range(B):
        acc = res.results[4 * b]["outT"].astype(np.float32)
        for g in range(1, 4):
            acc = acc + res.results[4 * b + g]["outT"].astype(np.float32)
        out[b] = acc.T + bo_full[None, :]
    return out


# revision 13
# speedup vs baseline: 1.5008x; 1.0484x over previous
# Causal self-attention on 8 TRN2 NeuronCores.
#
# Sharding (data + tensor parallel per the hint):
#   core c -> batch b = c // 4, head group g = c % 4 (4 heads of 64 dims = 256).
#   Wq/Wk/Wv split column-wise per head group; Wo row-wise. Each core emits a
#   partial [D, S] output in bf16; the host sums the 4 partials per batch
#   (the "all-reduce" of row-parallel sharding), transposes, and adds
#   bo' = bo + bv @ Wo.T (the V-bias commutes through softmax-normalize +
#   out-projection, so it is folded into the host-side bias).
#
# Device kernel (per core), all matmuls bf16 (PE streams 1 col/cycle for
# every dtype, so bf16 costs the same PE time as fp32 but halves DMA/SBUF
# and unlocks 2x DVE modes; tolerance is 2e-2, bf16 lands ~1e-3):
#   xT [D, S] resident in SBUF (bf16).
#   QT/KT [128 = 2 heads x 64, pair, S] = W x; bias added on DVE during the
#     PSUM->SBUF stage via per-partition tensor_scalar_add (no bias matmuls;
#     1/8 softmax scale folded into Wq/bq on the host).
#   V [S, 4 heads, 64+1] with a ones column (rowsum rides along in PV).
#   Attention is one global chunk pipeline across all (pair, q-block)
#   blocks: per chunk, a row-tiled pair of K=64 score matmuls (two heads
#   concurrently in the PE array), exp on ACT (PSUM->SBUF bf16), causal
#   mask multiply on GpSimd (diag chunks), then PV accumulation.
#   The PE stream is emitted with scores lookahead 1 (scores of chunk i+1
#   precede PV of chunk i) and a calibrated amount of "filler" matmuls
#   (projections for later blocks + out-projection of finished q-blocks)
#   between them, so the PE never idles waiting for ACT and the HAM clock
#   gate stays at 2.4 GHz. ACT's exp stream is the secondary resource
#   (~68us vs ~100us PE); fillers are deferred so the late, ACT-heavy
#   blocks still have PE work available.
#   Normalization: rowsums -> PE broadcast (K=1 matmul) -> DVE approx
#   reciprocal -> multiply (head 0 on DVE in place, head 1 via tmp + DMA to
#   partitions 64..127).
#   Out projection: 2 accumulating matmuls per [128,512] tile, staged
#   PSUM->SBUF bf16 alternating DVE/ACT, DMA'd out bf16 (no bias on device).

import os

import numpy as np

S = 2048
D = 1024
DL = 256  # local head dims (4 heads x 64)
NCORES = 8

_cache = {}
LAST_EXEC_TIME_NS = None
LAST_TRACE_PATH = None


DEBUG = os.environ.get("KERNEL_DEBUG", "0") == "1"


def _build_bass():
    from concourse import bacc
    import concourse.tile as tile
    import concourse.mybir as mybir
    from concourse.bass import ts, ds

    f32 = mybir.dt.float32
    bf16 = mybir.dt.bfloat16
    Exp = mybir.ActivationFunctionType.Exp

    nc = bacc.Bacc("TRN2", target_bir_lowering=False, debug=False)

    xT_d = nc.dram_tensor("xT", [D, S], bf16, kind="ExternalInput")
    wqT_d = nc.dram_tensor("wqT", [D, DL], bf16, kind="ExternalInput")
    wkT_d = nc.dram_tensor("wkT", [D, DL], bf16, kind="ExternalInput")
    wvT_d = nc.dram_tensor("wvT", [D, DL], bf16, kind="ExternalInput")
    woT_d = nc.dram_tensor("woT", [DL, D], bf16, kind="ExternalInput")
    bq_d = nc.dram_tensor("bq", [128, 2], f32, kind="ExternalInput")
    bk_d = nc.dram_tensor("bk", [128, 2], f32, kind="ExternalInput")
    mask_d = nc.dram_tensor("mask", [128, 128], bf16, kind="ExternalInput")
    out_d = nc.dram_tensor("outT", [D, S], bf16, kind="ExternalOutput")
    warm_d = nc.dram_tensor("warm", [2, 512], f32, kind="ExternalOutput")
    if DEBUG:
        qT_o = nc.dram_tensor("qT_o", [128, 2, S], bf16, kind="ExternalOutput")
        kT_o = nc.dram_tensor("kT_o", [128, 2, S], bf16, kind="ExternalOutput")
        v4_o = nc.dram_tensor("v4_o", [128, 16, 4, 65], bf16, kind="ExternalOutput")
        oT_o = nc.dram_tensor("oT_o", [128, 2, S], bf16, kind="ExternalOutput")

    with tile.TileContext(nc) as tc:
        with (
            tc.tile_pool(name="persist", bufs=1) as persist,
            tc.tile_pool(name="ptp", bufs=4) as ptp,
            tc.tile_pool(name="oup", bufs=2) as oup,
            tc.tile_pool(name="rbp", bufs=2) as rbp,
            tc.tile_pool(name="stp", bufs=3) as stp,
            tc.tile_pool(name="tbp", bufs=2) as tbp,
            tc.tile_pool(name="wsp", bufs=1) as wsp,
            tc.tile_pool(name="sc2", bufs=2, space="PSUM") as sc2,
            tc.tile_pool(name="mm", bufs=2, space="PSUM") as mm,
            tc.tile_pool(name="po", bufs=2, space="PSUM") as po,
        ):
            # ---- persistent SBUF tensors ----
            xT = persist.tile([128, 8, S], bf16, name="xT_sb")
            wqT = persist.tile([128, 8, DL], bf16, name="wqT_sb")
            wkT = persist.tile([128, 8, DL], bf16, name="wkT_sb")
            wvT = persist.tile([128, 8, DL], bf16, name="wvT_sb")
            woT = persist.tile([128, 2, D], bf16, name="woT_sb")
            bq = persist.tile([128, 2], f32, name="bq_sb")
            bk = persist.tile([128, 2], f32, name="bk_sb")
            mask = persist.tile([128, 128], bf16, name="mask_sb")
            ones_bf = persist.tile([128, 512], bf16, name="ones_bf")
            qT = persist.tile([128, 2, S], bf16, name="qT_sb")
            kT = persist.tile([128, 2, S], bf16, name="kT_sb")
            v4 = persist.tile([128, 16, 4, 65], bf16, name="v4_sb")
            oT = persist.tile([128, 2, S], bf16, name="oT_sb")

            # ---- input DMAs ----
            wq_r = wqT_d.ap().rearrange("(o p) f -> p o f", p=128)
            wk_r = wkT_d.ap().rearrange("(o p) f -> p o f", p=128)
            wv_r = wvT_d.ap().rearrange("(o p) f -> p o f", p=128)
            wo_r = woT_d.ap().rearrange("(o p) f -> p o f", p=128)
            x_r = xT_d.ap().rearrange("(o p) f -> p o f", p=128)
            # gpsimd starts earliest and is otherwise idle: constants there
            nc.gpsimd.memset(ones_bf[:], 1.0)
            nc.gpsimd.memset(v4[:, :, :, 64:65], 1.0)
            # sync ring, in first-use order: wq+x(qb0) gate the pre-phase,
            # wk/wv before the rest of x, wo (out-proj) last
            nc.sync.dma_start(wqT[:], wq_r)
            nc.sync.dma_start(xT[:, :, ts(0, 512)], x_r[:, :, ts(0, 512)])
            nc.sync.dma_start(wkT[:], wk_r)
            nc.sync.dma_start(wvT[:], wv_r)
            for tb in range(1, 4):
                nc.sync.dma_start(xT[:, :, ts(tb, 512)], x_r[:, :, ts(tb, 512)])
            nc.sync.dma_start(woT[:], wo_r)
            # scalar ring: only tiny constants (a big transfer here would
            # stall the ACT sequencer mid-dma_start and block the exps)
            nc.scalar.dma_start(bq[:], bq_d.ap())
            nc.scalar.dma_start(bk[:], bk_d.ap())
            nc.scalar.dma_start(mask[:], mask_d.ap())

            # ---- ACT table preload: dummy exp while DMAs stream ----
            wexp = wsp.tile([1, 512], f32, name="wexp")
            nc.scalar.activation(wexp[:], ones_bf[0:1, :], Exp)
            nc.sync.dma_start(warm_d.ap()[1:2, :], wexp[:])

            # ---- PE warmup: keep the array busy (and HAM warming) until
            # the first projection's inputs arrive (~6-7us) ----
            NWARM = 10
            psW = mm.tile([128, 512], f32, tag="mm", name="psW")
            for i in range(NWARM):
                nc.tensor.matmul(
                    psW,
                    lhsT=ones_bf[:, 0:128],
                    rhs=ones_bf[:],
                    start=(i == 0),
                    stop=(i == NWARM - 1),
                    skip_group_check=True,
                )
            wstg = wsp.tile([1, 512], f32, name="wstg")
            nc.vector.tensor_copy(wstg[:], psW[0:1, :])
            nc.sync.dma_start(warm_d.ap()[0:1, :], wstg[:])

            # ---- filler units: (est_pe_ns, emit_fn) ----
            def qk_proj_units(wsb, bvec, dst, t, qb):
                cell = {}

                def mk(mc):
                    def fn():
                        if mc == 0:
                            cell["ps"] = mm.tile(
                                [128, 512], f32, tag="mm", name="psqk"
                            )
                        nc.tensor.matmul(
                            cell["ps"],
                            lhsT=wsb[:, mc, ts(t, 128)],
                            rhs=xT[:, mc, ts(qb, 512)],
                            start=(mc == 0),
                            stop=(mc == 7),
                            skip_group_check=True,
                        )
                        if mc == 7:
                            nc.vector.tensor_scalar_add(
                                dst[:, t, ts(qb, 512)],
                                cell["ps"],
                                bvec[:, t : t + 1],
                            )

                    return (270, fn)

                return [mk(mc) for mc in range(8)]

            def v_proj_units(st):
                cell = {}

                def mk(mc):
                    def fn():
                        if mc == 0:
                            cell["ps"] = mm.tile(
                                [128, 512], f32, tag="mm", name="psv"
                            )
                        nc.tensor.matmul(
                            cell["ps"][:, 0:256],
                            lhsT=xT[:, mc, ts(st, 128)],
                            rhs=wvT[:, mc, :],
                            start=(mc == 0),
                            stop=(mc == 7),
                            skip_group_check=True,
                        )
                        if mc == 7:
                            nc.vector.tensor_copy(
                                v4[:, st, :, 0:64],
                                cell["ps"][:, 0:256].rearrange(
                                    "p (h d) -> p h d", h=4
                                ),
                            )

                    return (160, fn)

                return [mk(mc) for mc in range(8)]

            op_count = [0]

            def outproj_units(sb):
                units = []
                for jt in range(8):

                    def fn(jt=jt):
                        ps = mm.tile([128, 512], f32, tag="mm", name="psop")
                        for dchunk in range(2):
                            nc.tensor.matmul(
                                ps,
                                lhsT=woT[:, dchunk, ts(jt, 128)],
                                rhs=oT[:, dchunk, ts(sb, 512)],
                                start=(dchunk == 0),
                                stop=(dchunk == 1),
                                skip_group_check=True,
                            )
                        stg = stp.tile([128, 512], bf16, tag="st", name="stg")
                        if op_count[0] % 2 == 0:
                            nc.vector.tensor_copy(stg[:], ps)
                        else:
                            nc.scalar.copy(stg[:], ps)
                        op_count[0] += 1
                        nc.sync.dma_start(
                            out_d.ap()[ts(jt, 128), ts(sb, 512)], stg[:]
                        )

                    units.append((560, fn))
                return units

            filler = []  # list of (cost, fn), consumed front-first
            consumed = [0]

            def drain(budget_ns):
                spent = 0
                while filler and spent < budget_ns:
                    cost, fn = filler.pop(0)
                    fn()
                    consumed[0] += 1
                    spent += cost

            def drain_until(count):
                # force-consume prerequisite units: a block's scores may
                # never be emitted into the PE FIFO ahead of the filler
                # matmuls that produce its Q/K/V (in-order queue deadlock)
                while filler and consumed[0] < count:
                    cost, fn = filler.pop(0)
                    fn()
                    consumed[0] += 1

            # ---- attention chunk pipeline ----
            class Ch:
                __slots__ = (
                    "pair", "qb", "c", "w", "q0", "dc",
                    "first", "last", "ps2", "pt",
                )

            chunks = []
            for qb in range(4):
                for pair in range(2):
                    nch = 4 * qb + 4
                    for c in range(nch):
                        ch = Ch()
                        ch.pair, ch.qb, ch.c = pair, qb, c
                        dc = c - 4 * qb
                        ch.dc = dc
                        ch.q0 = 128 * dc if dc >= 0 else 0
                        ch.w = 512 - ch.q0
                        ch.first = c == 0
                        ch.last = c == nch - 1
                        chunks.append(ch)

            def emit_scores(ch):
                ps2 = sc2.tile([128, 2, 512], f32, tag="sc", name="ps2")
                for hh in (0, 1):
                    prow = slice(64 * hh, 64 * hh + 64)
                    nc.tensor.matmul(
                        ps2[:, hh, : ch.w],
                        lhsT=kT[prow, ch.pair, ts(ch.c, 128)],
                        rhs=qT[prow, ch.pair, ds(512 * ch.qb + ch.q0, ch.w)],
                        start=True,
                        stop=True,
                    )
                ch.ps2 = ps2

            def emit_exp(ch):
                pt = ptp.tile([128, 2, 512], bf16, tag="pt", name="pt")
                nc.scalar.activation(pt[:, :, : ch.w], ch.ps2[:, :, : ch.w], Exp)
                if ch.dc >= 0:
                    nc.gpsimd.tensor_mul(
                        pt[:, :, 0:128],
                        pt[:, :, 0:128],
                        mask[:, None, :].to_broadcast((128, 2, 128)),
                    )
                ch.pt = pt

            blk = {}

            def emit_pv(ch):
                if ch.first:
                    blk["psA"] = po.tile([128, 512], f32, tag="po", name="psA")
                    blk["psB"] = po.tile([128, 512], f32, tag="po", name="psB")
                for hh, psO in ((0, blk["psA"]), (1, blk["psB"])):
                    nc.tensor.matmul(
                        psO[0:65, ds(ch.q0, ch.w)],
                        lhsT=v4[:, ch.c, 2 * ch.pair + hh, :],
                        rhs=ch.pt[:, hh, : ch.w],
                        start=ch.first,
                        stop=ch.last,
                        skip_group_check=True,
                    )

            def emit_norm(pair, qb):
                psA, psB = blk["psA"], blk["psB"]
                oA = oup.tile([128, 512], bf16, tag="ou", name="oA")
                oB = oup.tile([128, 512], bf16, tag="ou", name="oB")
                nc.vector.tensor_copy(oA[0:65, :], psA[0:65, :])
                nc.vector.tensor_copy(oB[0:65, :], psB[0:65, :])
                psR = mm.tile([128, 512], f32, tag="mm", name="psR")
                nc.tensor.matmul(
                    psR[0:64, :],
                    lhsT=ones_bf[64:65, 0:64],
                    rhs=oA[64:65, :],
                    start=True,
                    stop=True,
                    skip_group_check=True,
                )
                rbA = rbp.tile([128, 512], f32, tag="rb", name="rbA")
                nc.vector.reciprocal_approx_fast(rbA[0:64, :], psR[0:64, :])
                psR2 = mm.tile([128, 512], f32, tag="mm", name="psR2")
                nc.tensor.matmul(
                    psR2[0:64, :],
                    lhsT=ones_bf[64:65, 0:64],
                    rhs=oB[64:65, :],
                    start=True,
                    stop=True,
                    skip_group_check=True,
                )
                rbB = rbp.tile([128, 512], f32, tag="rb", name="rbB")
                nc.vector.reciprocal_approx_fast(rbB[0:64, :], psR2[0:64, :])
                nc.gpsimd.tensor_mul(
                    oT[0:64, pair, ts(qb, 512)], oA[0:64, :], rbA[0:64, :]
                )
                tmpB = tbp.tile([128, 512], bf16, tag="tb", name="tmpB")
                nc.gpsimd.tensor_mul(tmpB[0:64, :], oB[0:64, :], rbB[0:64, :])
                nc.sync.dma_start(oT[64:128, pair, ts(qb, 512)], tmpB[0:64, :])

            # ---- pre-phase: minimal projections for (pair0, qb0, chunk0) ----
            for u in qk_proj_units(wqT, bq, qT, 0, 0):
                u[1]()
            for u in qk_proj_units(wkT, bk, kT, 0, 0):
                u[1]()
            for u in v_proj_units(0):
                u[1]()

            # ---- filler schedule (dependency-ordered, deliberately
            # back-loaded: projections just-in-time, out-proj deferred to
            # the late ACT-heavy blocks) ----
            # consumed during (p0,qb0): V st1-3 (gated per-chunk by
            # pv_prereq below), then Q/K t1 qb0 (needed by p1,qb0)
            for st in range(1, 4):
                filler += v_proj_units(st)
            filler += qk_proj_units(wqT, bq, qT, 1, 0)
            filler += qk_proj_units(wkT, bk, kT, 1, 0)
            # during (p1,qb0): Q/K t0 qb1 + V st4-7 (needed by qb1)
            filler += qk_proj_units(wqT, bq, qT, 0, 1)
            filler += qk_proj_units(wkT, bk, kT, 0, 1)
            for st in range(4, 8):
                filler += v_proj_units(st)
            # during (p0,qb1): Q/K t1 qb1
            filler += qk_proj_units(wqT, bq, qT, 1, 1)
            filler += qk_proj_units(wkT, bk, kT, 1, 1)
            # during (p1,qb1): Q/K t0 qb2 + V st8-11
            filler += qk_proj_units(wqT, bq, qT, 0, 2)
            filler += qk_proj_units(wkT, bk, kT, 0, 2)
            for st in range(8, 12):
                filler += v_proj_units(st)
            # during (p0,qb2): Q/K t1 qb2
            filler += qk_proj_units(wqT, bq, qT, 1, 2)
            filler += qk_proj_units(wkT, bk, kT, 1, 2)
            # during (p1,qb2): Q/K t0 qb3 + V st12-15
            filler += qk_proj_units(wqT, bq, qT, 0, 3)
            filler += qk_proj_units(wkT, bk, kT, 0, 3)
            for st in range(12, 16):
                filler += v_proj_units(st)
            # during (p0,qb3): Q/K t1 qb3
            filler += qk_proj_units(wqT, bq, qT, 1, 3)
            filler += qk_proj_units(wkT, bk, kT, 1, 3)
            # out-proj units are appended as their q-block completes

            # units that must be consumed before each block's first scores
            # (cumulative position in the dependency-ordered filler list);
            # blocks in order (p,qb): (1,0),(0,1),(1,1),(0,2),(1,2),(0,3),(1,3)
            prereq = {
                (1, 0): 40,
                (0, 1): 88,
                (1, 1): 104,
                (0, 2): 152,
                (1, 2): 168,
                (0, 3): 216,
                (1, 3): 232,
            }
            # V st1-3 sit at filler positions 0..23; (p0,qb0) chunk c's PV
            # needs V st c emitted first (PE FIFO would deadlock otherwise)
            pv_prereq = {(0, 0, 1): 8, (0, 0, 2): 16, (0, 0, 3): 24}

            emit_scores(chunks[0])
            emit_exp(chunks[0])
            nchunks_total = len(chunks)
            for i, ch in enumerate(chunks):
                if i + 1 < len(chunks):
                    nxt = chunks[i + 1]
                    if nxt.first and (nxt.pair, nxt.qb) in prereq:
                        drain_until(prereq[(nxt.pair, nxt.qb)])
                    emit_scores(nxt)
                    emit_exp(nxt)
                # keep PE fed while ACT computes exp(ch): spread the
                # remaining filler evenly over the remaining chunks so the
                # late ACT-heavy blocks never starve the PE
                remaining = sum(c for c, _ in filler)
                left = nchunks_total - i
                drain(max(300 + 0.45 * ch.w, 1.05 * remaining / left))
                emit_pv(ch)
                if ch.last:
                    emit_norm(ch.pair, ch.qb)
                    if ch.pair == 1:
                        filler += outproj_units(ch.qb)

            # tail: whatever out-proj work is left (at least sb=3)
            while filler:
                filler.pop(0)[1]()

            if DEBUG:
                nc.sync.dma_start(qT_o.ap(), qT[:])
                nc.sync.dma_start(kT_o.ap(), kT[:])
                nc.sync.dma_start(v4_o.ap(), v4[:])
                nc.sync.dma_start(oT_o.ap(), oT[:])

    nc.compile()
    return nc


def _get_bass():
    if "nc" not in _cache:
        _cache["nc"] = _build_bass()
    return _cache["nc"]


def _shard_inputs(x, Wq, bq, Wk, bk, Wv, bv, Wo, bo):
    import ml_dtypes

    bft = ml_dtypes.bfloat16
    x = np.asarray(x, dtype=np.float32)
    Wq = np.asarray(Wq, dtype=np.float32)
    Wk = np.asarray(Wk, dtype=np.float32)
    Wv = np.asarray(Wv, dtype=np.float32)
    Wo = np.asarray(Wo, dtype=np.float32)
    bq = np.asarray(bq, dtype=np.float32)
    bk = np.asarray(bk, dtype=np.float32)

    kk = np.arange(128)[:, None]
    qq = np.arange(128)[None, :]
    mask128 = (kk <= qq).astype(bft)

    xT = [np.ascontiguousarray(x[b].T).astype(bft) for b in range(x.shape[0])]
    in_maps = []
    for c in range(NCORES):
        b, g = divmod(c, 4)
        sl = slice(DL * g, DL * (g + 1))
        in_maps.append(
            {
                "xT": xT[b],
                "wqT": (np.ascontiguousarray(Wq[sl].T) * 0.125).astype(bft),
                "wkT": np.ascontiguousarray(Wk[sl].T).astype(bft),
                "wvT": np.ascontiguousarray(Wv[sl].T).astype(bft),
                "woT": np.ascontiguousarray(Wo[:, sl].T).astype(bft),
                "bq": np.ascontiguousarray(
                    (bq[sl] * 0.125).reshape(2, 128).T
                ),
                "bk": np.ascontiguousarray(bk[sl].reshape(2, 128).T),
                "mask": mask128,
            }
        )
    return in_maps


def kernel(x, Wq, bq, Wk, bk, Wv, bv, Wo, bo):
    global LAST_EXEC_TIME_NS, LAST_TRACE_PATH
    from concourse.bass_utils import run_bass_kernel_spmd

    nc = _get_bass()
    in_maps = _shard_inputs(x, Wq, bq, Wk, bk, Wv, bv, Wo, bo)

    trace = os.environ.get("KERNEL_TRACE", "0") == "1"
    res = run_bass_kernel_spmd(
        nc, in_maps, core_ids=list(range(NCORES)), trace=trace
    )
    LAST_EXEC_TIME_NS = res.exec_time_ns
    if res.instructions_and_trace is not None:
        LAST_TRACE_PATH = res.instructions_and_trace[1]

    bo_full = (
        np.asarray(bo, np.float64)
        + np.asarray(bv, np.float64) @ np.asarray(Wo, np.float64).T
    ).astype(np.float32)

    B = 2
    out = np.empty((B, S, D), dtype=np.float32)
    for b in range(B):
        acc = res.results[4 * b]["outT"].astype(np.float32)
        for g in range(1, 4):
            acc = acc + res.results[4 * b + g]["outT"].astype(np.float32)
        out[b] = acc.T + bo_full[None, :]
    return out
